# revision 1
# baseline (speedup 1.0000x reference)
"""Trainium2 Bass kernel for nn_Model_11888469475981 (pooling).

Reference semantics (per sample n, channel c):
  x_p = zeropad3d(x, W:(1,2), H:(1,1), D:(0,1))           # (17, 66, 259)
  rows = x_p rows along W (each length Wp=259), K=3 S=2 maxpool w/ indices,
  softsign, max-unpool scatter, add x_p, mean over padded D (17).

Key restructure (exact, no gather/scatter):
  For a padded row A[0..258], position w receives softsign(A[w]) iff some
  window picks w as its (first-occurrence) argmax. With L[w] = [A[w] > A[w-1]]
  and e1[m] = [A[2m] >= A[2m+2]]:
    odd w=2m+1 : mask = L[2m+1] * (1 - L[2m+2])
    even w=2m  : mask = max( (1-L[2m+1])*e1[m],  (1-e1[m-1])*L[2m] )
  fused[w] = A[w] * (1 + mask[w] * 1/(1+|A[w]|))
  out[h, w] = (1/17) * sum_d fused[d, h, w]   (padded D slab and padded H/W
  rows are exactly zero and are written as zeros / via the 1/17 weight).

Layout per core (1 sample): per channel c, one SBUF tile [128, 8*264]:
  partition p = d*8 + hg  (d in 0..15, hg = h//8), free = hs-slot (h%8) * 264.
  Slot: [2 guard][259 padded-W cols][3 guard], real x at cols 3..258.
  Depth-mean via PE matmul with lhsT W8[p, m] = (1/17)*[p%8 == m] -> psum[8,264].
"""

import numpy as np

import concourse.bass as bass
import concourse.mybir as mybir
from concourse import bacc
from concourse.tile import TileContext
from concourse.bass_utils import run_bass_kernel_spmd

N_CORES = 8
C, D, H, W = 32, 16, 64, 256
HP, WP = 66, 259
SLOT = 264
NS = 8              # h-subslots per partition
FREE = NS * SLOT
DSLOT = 132         # dense (per-window-index m) slot width
DFREE = NS * DSLOT
F32 = mybir.dt.float32
Alu = mybir.AluOpType
Act = mybir.ActivationFunctionType


def _fullw(t, c0, cnt):
    return t[:].rearrange("p (s w) -> p s w", s=NS)[:, :, c0:c0 + cnt]


def _dense(t, c0, cnt):
    return t[:].rearrange("p (s w) -> p s w", s=NS)[:, :, c0:c0 + cnt]


def _v2(t):
    return t[:].rearrange("p (s w2 two) -> p s w2 two", s=NS, two=2)


def _ev(t, mshift, cnt):
    # even padded-w columns: col = 2 + 2*(m + mshift), m in [0, cnt)
    return _v2(t)[:, :, 1 + mshift:1 + mshift + cnt, 0]


def _od(t, mshift, cnt):
    # odd padded-w columns: col = 3 + 2*(m + mshift), m in [0, cnt)
    return _v2(t)[:, :, 1 + mshift:1 + mshift + cnt, 1]


def build_nc():
    # Bacc: its finalize() runs the wait-splitting / legalization passes
    # (TRN2 allows at most 1 sync wait per instruction).
    nc = bacc.Bacc()
    x_ext = nc.declare_dram_parameter("x", [C, D, H, W], F32, isOutput=False)
    w8_ext = nc.declare_dram_parameter("w8", [128, 8], F32, isOutput=False)
    out_ext = nc.declare_dram_parameter("out", [C, HP, WP], F32, isOutput=True)

    with TileContext(nc) as tc:
        with tc.tile_pool(name="main", bufs=1) as pool, \
             tc.tile_pool(name="psum", bufs=2, space="PSUM") as psum_pool:
            a_ts = [pool.tile([128, FREE], F32, tag=f"a{i}", name=f"a{i}") for i in range(3)]
            f_ts = [pool.tile([128, FREE], F32, tag=f"fu{i}", name=f"fu{i}") for i in range(2)]
            m2_ts = [pool.tile([128, FREE], F32, tag=f"m2{i}", name=f"m2{i}") for i in range(2)]
            r_ts = [pool.tile([128, FREE], F32, tag=f"r{i}", name=f"r{i}") for i in range(2)]
            ab_t = pool.tile([128, FREE], F32, tag="abs", name="abs")
            ln_t = pool.tile([128, FREE], F32, tag="lnt", name="lnt")
            l_t = pool.tile([128, FREE], F32, tag="lcmp", name="lcmp")
            e1_t = pool.tile([128, DFREE], F32, tag="e1", name="e1")
            to_t = pool.tile([128, DFREE], F32, tag="todd", name="todd")
            fe_t = pool.tile([128, DFREE], F32, tag="fev", name="fev")
            le_t = pool.tile([128, DFREE], F32, tag="lev", name="lev")
            mk_t = pool.tile([128, DFREE], F32, tag="mask", name="mask")
            w8_t = pool.tile([128, 8], F32, tag="w8", name="w8")
            o_ts = [pool.tile([8, NS * WP], F32, tag=f"o{i}", name=f"o{i}")
                    for i in range(2)]
            z_t = pool.tile([32, 2 * WP], F32, tag="zrow", name="zrow")

            # one-time init: zero guards (and any never-written-but-read cols).
            # Memsets run on DVE so downstream DVE/PE consumers do not need an
            # extra cross-engine semaphore wait (walrus caps waits per inst).
            for t in a_ts + f_ts + m2_ts + r_ts:
                nc.vector.memset(t[:], 0.0)
            nc.vector.memset(l_t[:], 0.0)
            nc.vector.memset(e1_t[:], 0.0)
            nc.gpsimd.memset(z_t[:], 0.0)
            nc.sync.dma_start(out=w8_t[:], in_=w8_ext[:, :])

            # padded-H border rows (h'=0 and h'=65) for every channel: zeros
            nc.sync.dma_start(
                out=bass.AP(out_ext, 0, [[HP * WP, C], [65 * WP, 2], [1, WP]]),
                in_=z_t[:].rearrange("p (a w) -> p a w", w=WP),
            )

            for c in range(C):
                a_t = a_ts[c % 3]
                F_t = f_ts[c % 2]
                m2_t = m2_ts[c % 2]
                r_t = r_ts[c % 2]

                # load channel: rows r=(d*64+h) -> partition p=d*8+h//8, slot h%8
                av = a_t[:].rearrange("p (s w) -> p s w", s=NS)
                nc.sync.dma_start(
                    out=av[:, :, 3:259],
                    in_=bass.AP(
                        x_ext,
                        c * D * H * W,
                        [[2048, 128], [256, NS], [1, W]],
                    ),
                )

                # L[w] = A[w] > A[w-1], w=0..258 (cols 2..260)
                nc.vector.tensor_tensor(
                    _fullw(l_t, 2, 259), _fullw(a_t, 2, 259), _fullw(a_t, 1, 259),
                    Alu.is_gt)
                # e1[m] = A[2m] >= A[2m+2], m=0..129
                nc.vector.tensor_tensor(
                    _dense(e1_t, 2, 130), _ev(a_t, 0, 130), _ev(a_t, 1, 130),
                    Alu.is_ge)
                # odd mask: todd[m] = (L[2m+2]==0) * L[2m+1], m=0..128
                nc.vector.scalar_tensor_tensor(
                    _dense(to_t, 2, 129), _ev(l_t, 1, 129), 0.0, _od(l_t, 0, 129),
                    Alu.is_equal, Alu.mult)
                # even "first": fe[m] = (L[2m+1]==0) * e1[m], m=0..129
                nc.vector.scalar_tensor_tensor(
                    _dense(fe_t, 2, 130), _od(l_t, 0, 130), 0.0, _dense(e1_t, 2, 130),
                    Alu.is_equal, Alu.mult)
                # even "last": le[m] = (e1[m-1]==0) * L[2m], m=0..129
                nc.vector.scalar_tensor_tensor(
                    _dense(le_t, 2, 130), _dense(e1_t, 1, 130), 0.0, _ev(l_t, 0, 130),
                    Alu.is_equal, Alu.mult)
                # even mask = max(first, last)
                nc.vector.tensor_tensor(
                    _dense(mk_t, 2, 130), _dense(fe_t, 2, 130), _dense(le_t, 2, 130),
                    Alu.max)

                # softsign reciprocal on ACT: r = 1/(1+|A|) = sigmoid(-ln|A|).
                # Only real cols 3..258; r at pad cols stays 0 from the
                # one-time memset (m2 = mask*0 = 0 there, and A=0 -> F=0).
                nc.scalar.activation(_fullw(ab_t, 3, 256), _fullw(a_t, 3, 256),
                                     Act.Abs)
                nc.scalar.activation(_fullw(ln_t, 3, 256), _fullw(ab_t, 3, 256),
                                     Act.Ln)
                nc.scalar.activation(_fullw(r_t, 3, 256), _fullw(ln_t, 3, 256),
                                     Act.Sigmoid, scale=-1.0)

                # m2 = mask * r  (parity-split writes)
                nc.vector.tensor_tensor(
                    _od(m2_t, 0, 129), _dense(to_t, 2, 129), _od(r_t, 0, 129),
                    Alu.mult)
                nc.vector.tensor_tensor(
                    _ev(m2_t, 0, 130), _dense(mk_t, 2, 130), _ev(r_t, 0, 130),
                    Alu.mult)
                # fused = (m2 + 1) * A
                nc.vector.scalar_tensor_tensor(
                    _fullw(F_t, 2, 260), _fullw(m2_t, 2, 260), 1.0,
                    _fullw(a_t, 2, 260), Alu.add, Alu.mult)

                # depth-sum via PE: psum[hg, w] = sum_d F[(d,hg), w], then
                # ScalarE evacuates PSUM->SBUF applying the 1/17 mean scale.
                Fv = F_t[:].rearrange("p (s w) -> p s w", s=NS)
                osb = o_ts[c % 2]
                ov = osb[:].rearrange("p (s w) -> p s w", s=NS)
                for half in range(2):
                    ps = psum_pool.tile([8, 4 * 512], F32, tag="ps",
                                        name=f"ps_{c}_{half}")
                    psv = ps[:].rearrange("p (s w) -> p s w", s=4)
                    for k in range(4):
                        hs = half * 4 + k
                        nc.tensor.matmul(psv[:, k, 0:SLOT], w8_t[:, 0:8],
                                         Fv[:, hs, :], start=True, stop=True)
                    nc.scalar.mul(ov[:, 4 * half:4 * half + 4, :],
                                  psv[:, :, 2:261], 1.0 / 17.0)
                nc.sync.dma_start(
                    out=bass.AP(out_ext, (c * HP + 1) * WP,
                                [[8 * WP, 8], [WP, NS], [1, WP]]),
                    in_=ov[:, :, :],
                )
    nc.finalize()
    return nc


_CACHE: dict = {}


def _get_nc():
    if "nc" not in _CACHE:
        _CACHE["nc"] = build_nc()
    return _CACHE["nc"]


def make_in_maps(x: np.ndarray):
    w8 = np.zeros((128, 8), np.float32)
    w8[np.arange(128), np.arange(128) % 8] = 1.0
    return [
        {"x": np.ascontiguousarray(x[i]), "w8": w8}
        for i in range(N_CORES)
    ]


def kernel(**inputs) -> np.ndarray:
    x = np.ascontiguousarray(np.asarray(inputs["x"], dtype=np.float32))
    assert x.shape == (N_CORES, C, D, H, W), x.shape
    nc = _get_nc()
    res = run_bass_kernel_spmd(nc, make_in_maps(x), list(range(N_CORES)))
    return np.stack([res.results[i]["out"] for i in range(N_CORES)], axis=0)



# revision 2
# speedup vs baseline: 1.0036x; 1.0036x over previous
"""Trainium2 Bass kernel for nn_Model_11888469475981 (pooling) — v8 (fp16, parity-split).

Per-core (1 sample): zeropad3d -> maxpool1d(K=3,S=2) w/ indices -> softsign
-> max-unpool scatter -> + x_p -> mean over padded depth (17).

The whole elementwise pipeline runs on fp16 copies of x (loaded via a
GPSIMD casting DMA — only gpsimd DMAs may cast): picks, softsign values
and the A-part depth-sum all use the same fp16 values, so the kernel is
self-consistent; vs the fp32 reference this costs L2 ~ 2.4e-3 (measured
in numpy), well under the 2e-2 gate. Every DVE TensorTensor is 2-byte
packed -> 2x perf mode; both matmuls are fp16 at 1 cycle/row.

Value-vs-max mask formulation (padded row A[0..258], window m =
{2m, 2m+1, 2m+2}; first-occurrence argmax; fp16 ties break toward the
earlier element exactly like the reference):
  R[m] = max(A[2m], A[2m+1])        P[m] = max(R[m], A[2m+2])
  FE[m] = A[2m]   >= P[m]    (window m picks elem 0)
  TO[m] = A[2m+1] >= P[m]    (window m picks elem 1)
  LE[m] = A[2m]   >  R[m-1]  (window m-1 picks elem 2)
Softsign via one ACT table set (natural_log_exp_and_others has
Abs+Ln+Exp+Copy -> exactly 1 table load):
  RC = exp(-ln(1 + |P|)) = 1/(1+|P|);  SA = P*RC = softsign(P)
  m2_od[m] = TO*SA[m];  m2_ev[m] = (FE*SA[m]) | (LE*SA[m-1])  (bitwise or:
  operands are disjoint or bit-identical).
Depth mean: two PSUM-accumulated fp16 matmuls per slot (A + m2, one-hot
1/1 weights), evac applies the exact fp32 *1/17 (ACT half, Pool half).
Matmuls are exactly 256 wide covering real cols w=1..256; the zero pad
columns/rows of the output are pre-zeroed by two strided DMAs (w-pad
trick: [h,257],[h,258],[h+1,0] are contiguous in DRAM).

Layout per channel: A fp16 [128, 8*264]; partition p = d*8 + h//8, slot
s = h%8: [2 guard][w=0..258 at cols 2..260][3 guard]. Window-domain
dense fp16 tiles 8*136 (R/SA carry a leading guard col: R guard=6e4 so
LE[0]=0, SA guard=0). m2 fp16: 8*272 = [od at m | ev at 137+m] per slot;
its matmul reads W-order w=1..256 via raw AP [[1,128],[137? ->138,2]]
(od[m], ev[m+1] pairs).
"""

import numpy as np

import concourse.bass as bass
import concourse.mybir as mybir
from concourse import bacc
from concourse.tile import TileContext
from concourse.bass_utils import run_bass_kernel_spmd

N_CORES = 8
C, D, H, W = 32, 16, 64, 256
HP, WP = 66, 259
SLOT = 264
NS = 8
FREE = NS * SLOT
DS = 136
DFREE = NS * DS
M2S = 2 * DS

F32 = mybir.dt.float32
F16 = mybir.dt.float16
U16 = mybir.dt.uint16
Alu = mybir.AluOpType
Act = mybir.ActivationFunctionType


def _slots(t):
    return t[:].rearrange("p (s w) -> p s w", s=NS)


def _aev(a_t, mshift, cnt):
    v = a_t[:].rearrange("p (s m two) -> p s m two", s=NS, two=2)
    return v[:, :, 1 + mshift:1 + mshift + cnt, 0]


def _aod(a_t, mshift, cnt):
    v = a_t[:].rearrange("p (s m two) -> p s m two", s=NS, two=2)
    return v[:, :, 1 + mshift:1 + mshift + cnt, 1]


def _d(t, c0, cnt):
    return t[:].rearrange("p (s w) -> p s w", s=NS)[:, :, c0:c0 + cnt]


def build_nc():
    nc = bacc.Bacc()
    x_ext = nc.declare_dram_parameter("x", [C, D, H, W], F32, isOutput=False)
    w8_ext = nc.declare_dram_parameter("w8", [128, 8], F16, isOutput=False)
    out_ext = nc.declare_dram_parameter("out", [C, HP, WP], F32, isOutput=True)

    with TileContext(nc) as tc:
        with tc.tile_pool(name="main", bufs=1) as pool, \
             tc.tile_pool(name="psum", bufs=2, space="PSUM") as psum_pool:
            NA = 4
            ND = 4
            a_ts = [pool.tile([128, FREE], F32, tag=f"a{i}", name=f"a{i}")
                    for i in range(4)]
            ap_ts = [pool.tile([128, NS * 268], F16, tag=f"ap{i}", name=f"ap{i}")
                     for i in range(ND)]
            r_ts = [pool.tile([128, DFREE], F16, tag=f"r{i}", name=f"r{i}")
                    for i in range(ND)]
            p_ts = [pool.tile([128, DFREE], F16, tag=f"p{i}", name=f"p{i}")
                    for i in range(ND)]
            bp_ts = [pool.tile([128, DFREE], F16, tag=f"bp{i}", name=f"bp{i}")
                     for i in range(3)]
            ln_ts = [pool.tile([128, DFREE], F16, tag=f"ln{i}", name=f"ln{i}")
                     for i in range(3)]
            rc_ts = [pool.tile([128, DFREE], F16, tag=f"rc{i}", name=f"rc{i}")
                     for i in range(ND)]
            fe_ts = [pool.tile([128, DFREE], F16, tag=f"fe{i}", name=f"fe{i}")
                     for i in range(3)]
            to_ts = [pool.tile([128, DFREE], F16, tag=f"to{i}", name=f"to{i}")
                     for i in range(3)]
            le_ts = [pool.tile([128, DFREE], F16, tag=f"le{i}", name=f"le{i}")
                     for i in range(3)]
            sa_ts = [pool.tile([128, DFREE], F16, tag=f"sa{i}", name=f"sa{i}")
                     for i in range(3)]
            t2_ts = [pool.tile([128, DFREE], F16, tag=f"t2{i}", name=f"t2{i}")
                     for i in range(3)]
            m2_ts = [pool.tile([128, NS * M2S], F16, tag=f"m2{i}", name=f"m2{i}")
                     for i in range(3)]
            o_ts = [pool.tile([8, NS * 256], F32, tag=f"o{i}", name=f"o{i}")
                    for i in range(3)]
            z_t = pool.tile([32, 2 * WP], F32, tag="zrow", name="zrow")
            w8_t = pool.tile([128, 8], F16, tag="w8", name="w8")

            # ---- one-time init ------------------------------------------
            from concourse.hw_specs import get_activation_tables
            tab_names = list(get_activation_tables(nc.m.arch).keys())
            set_id = tab_names.index("natural_log_exp_and_others")
            nc.scalar.add_instruction(mybir.InstLoadActFuncSet(
                name=nc.get_next_instruction_name(),
                act_func_set_id=set_id, ins=[], outs=[]))
            nc.sync.dma_start(out=w8_t[:], in_=w8_ext[:, :])
            nc.gpsimd.memset(z_t[:], 0.0)
            for t in a_ts:
                av = _slots(t)
                nc.vector.memset(av[:, :, 0:3], 0.0)
                nc.vector.memset(av[:, :, 259:264], 0.0)
            for t in r_ts:
                nc.vector.memset(_d(t, 0, 1), 60000.0)
            for t in sa_ts:
                nc.vector.memset(_d(t, 0, 1), 0.0)
            for t in m2_ts:
                nc.vector.memset(t[:], 0.0)

            # padded-H border rows (h'=0 and h'=65): zeros for every channel
            nc.sync.dma_start(
                out=bass.AP(out_ext, 0, [[HP * WP, C], [65 * WP, 2], [1, WP]]),
                in_=z_t[:].rearrange("p (a w) -> p a w", w=WP),
            )
            # W-pad cols (w'=0,257,258, rows 1..64): [h,257],[h,258],[h+1,0]
            # are contiguous in DRAM -> one 3-wide strided DMA per channel.
            nc.sync.dma_start(
                out=bass.AP(out_ext, 257, [[HP * WP, C], [WP, 65], [1, 3]]),
                in_=z_t[:][:, 0:195].rearrange("p (a w) -> p a w", w=3),
            )

            for c in range(C):
                a_t = a_ts[c % NA]
                av = _slots(a_t)
                nc.sync.dma_start(
                    out=av[:, :, 3:259],
                    in_=bass.AP(x_ext, c * D * H * W,
                                [[2048, 128], [256, NS], [1, W]]),
                )
                r_t, p_t = r_ts[c % ND], p_ts[c % ND]
                bp, ln_t, rc = bp_ts[c % 3], ln_ts[c % 3], rc_ts[c % ND]
                fe, to, le = fe_ts[c % 3], to_ts[c % 3], le_ts[c % 3]
                sa, t2, m2 = sa_ts[c % 3], t2_ts[c % 3], m2_ts[c % 3]
                apar = ap_ts[c % ND]

                # parity-split cast copy (fp32 A -> dense fp16 [od|ev] planes)
                # out[p, s, two(od@0/ev@134), m=0..130]; in cols od=3+2m, ev=2+2m
                aap = a_t[:]
                pap = apar[:]
                nc.vector.tensor_copy(
                    bass.AP(pap.tensor, pap.offset,
                            [list(pap.ap)[0], [268, NS], [1, 131]]),
                    bass.AP(aap.tensor, aap.offset + 3,
                            [list(aap.ap)[0], [SLOT, NS], [2, 131]]))
                nc.scalar.activation(
                    bass.AP(pap.tensor, pap.offset + 134,
                            [list(pap.ap)[0], [268, NS], [1, 131]]),
                    bass.AP(aap.tensor, aap.offset + 2,
                            [list(aap.ap)[0], [SLOT, NS], [2, 131]]),
                    Act.Copy)
                apv = apar[:].rearrange("p (s two m) -> p s two m", s=NS, two=2)

                def _od16(m0, cnt):
                    return apv[:, :, 0, m0:m0 + cnt]

                def _ev16(m0, cnt):
                    return apv[:, :, 1, m0:m0 + cnt]

                # R[1+m] = max(A_ev[m], A_od[m]); P[m] = max(R[m], A_ev[m+1])
                nc.vector.tensor_tensor(
                    _d(r_t, 1, 130), _ev16(0, 130), _od16(0, 130), Alu.max)
                nc.vector.tensor_tensor(
                    _d(p_t, 0, 130), _d(r_t, 1, 130), _ev16(1, 130), Alu.max)

                # softsign reciprocal: RC = exp(-ln(1+|P|)), one table set
                nc.scalar.activation(_d(bp, 0, 130), _d(p_t, 0, 130), Act.Abs)
                nc.scalar.activation(_d(ln_t, 0, 130), _d(bp, 0, 130),
                                     Act.Ln, bias=1.0)
                nc.scalar.activation(_d(rc, 0, 130), _d(ln_t, 0, 130),
                                     Act.Exp, scale=-1.0)

                # masks from value-vs-max comparisons (fp16 2x on DVE)
                nc.vector.tensor_tensor(
                    _d(fe, 0, 130), _ev16(0, 130), _d(p_t, 0, 130),
                    Alu.is_ge)
                nc.vector.tensor_tensor(
                    _d(to, 0, 129), _od16(0, 129), _d(p_t, 0, 129),
                    Alu.is_ge)
                nc.vector.tensor_tensor(
                    _d(le, 0, 130), _ev16(0, 130), _d(r_t, 0, 130),
                    Alu.is_gt)
                # SA[1+m] = P*RC (softsign of the pooled max)
                nc.vector.tensor_tensor(
                    _d(sa, 1, 130), _d(p_t, 0, 130), _d(rc, 0, 130), Alu.mult)
                # m2 assembly; od half at cols 0.., ev at 137+m
                m2v = m2[:].rearrange("p (s w) -> p s w", s=NS)
                m2_od = m2v[:, :, 0:129]
                m2_ev = m2v[:, :, 137:267]
                nc.vector.tensor_tensor(
                    m2_od, _d(to, 0, 129), _d(sa, 1, 129), Alu.mult)
                nc.vector.tensor_tensor(
                    m2_ev, _d(fe, 0, 130), _d(sa, 1, 130), Alu.mult)
                # t2 = LE*SA[m-1] on Pool (mult is gpsimd-legal)
                nc.gpsimd.tensor_tensor(
                    _d(t2, 0, 130), _d(le, 0, 130), _d(sa, 0, 130), Alu.mult)
                nc.vector.tensor_tensor(
                    m2_ev.bitcast(U16), m2_ev.bitcast(U16),
                    _d(t2, 0, 130).bitcast(U16), Alu.bitwise_or)

                # depth-sum matmuls (256 wide = real cols w=1..256) + evac
                m2ap = m2[:]
                osb = o_ts[c % 3]
                ov = osb[:].rearrange("p (s w) -> p s w", s=NS)
                ps = psum_pool.tile([8, NS * 256], F32, tag="ps",
                                    name=f"ps_{c}")
                psv = ps[:].rearrange("p (s w) -> p s w", s=NS)
                for hs in range(NS):
                    nc.tensor.matmul(
                        psv[:, hs, :], w8_t[:, 0:8],
                        bass.AP(pap.tensor, pap.offset + hs * 268,
                                [list(pap.ap)[0], [1, 128], [135, 2]]),
                        start=True, stop=False)
                    nc.tensor.matmul(
                        psv[:, hs, :], w8_t[:, 0:8],
                        bass.AP(m2ap.tensor, m2ap.offset + hs * M2S,
                                [list(m2ap.ap)[0], [1, 128], [138, 2]]),
                        start=False, stop=True)
                # evac *1/17 (ACT: gpsimd cannot access PSUM)
                nc.scalar.mul(ov[:, :, :], psv[:, :, :], 1.0 / 17.0)
                nc.sync.dma_start(
                    out=bass.AP(out_ext, (c * HP + 1) * WP + 1,
                                [[8 * WP, 8], [WP, NS], [1, 256]]),
                    in_=ov[:, :, :],
                )
    nc.finalize()
    return nc


_CACHE: dict = {}


def _get_nc():
    if "nc" not in _CACHE:
        _CACHE["nc"] = build_nc()
    return _CACHE["nc"]


def make_in_maps(x: np.ndarray):
    w8 = np.zeros((128, 8), np.float16)
    w8[np.arange(128), np.arange(128) % 8] = 1.0
    return [
        {"x": np.ascontiguousarray(x[i]), "w8": w8}
        for i in range(N_CORES)
    ]


def kernel(**inputs) -> np.ndarray:
    x = np.ascontiguousarray(np.asarray(inputs["x"], dtype=np.float32))
    assert x.shape == (N_CORES, C, D, H, W), x.shape
    nc = _get_nc()
    res = run_bass_kernel_spmd(nc, make_in_maps(x), list(range(N_CORES)))
    return np.stack([res.results[i]["out"] for i in range(N_CORES)], axis=0)
